# revision 38
# baseline (speedup 1.0000x reference)
"""Two-layer GAT (4-head then 1-head) on 8 NeuronCores.

Layer 0 (K2): dst nodes are packed into 392 load-balanced windows of 128
(LPT/snake assignment equalizes edge counts -> minimal chunk padding);
per window the segment softmax + weighted aggregation run as one-hot-
selection matmuls on the tensor engine (one-hot built on-device via
is_equal against an iota table), with the per-edge alpha weighting done
by a single 2x-mode DVE multiply in an interleaved (chunk, feat, head)
layout.  Normalization is folded into per-head scalar-engine copies with
per-partition scale; L1's bias+relu folds into the post-transpose copy.

Layer 1 (K3): dst nodes are degree-sorted into windows of 128 with one
dst per partition and its edges along the free axis (per-window static
max degree).  Aggregation is then a per-partition tensor_reduce on DVE:
no matmuls, no one-hot, and er1[dst] rides the activation bias port.

Three SPMD launches (bf16 data path, f32 accumulation):
  K1: h|el|er = x @ [W0^T | vl0^T | vr0^T]   (node-sharded)
  K2: L0 edge phase + relu + L1 node matmul -> g|el1|er1
  K3: L1 edge phase -> output

Between launches the host performs pure index gathers (edge-ordered
copies of device-computed tables); all floating-point math runs on
device.
"""
import os
import sys
import types

sys.path.insert(0, "/opt/trn_rl_repo")

import numpy as np

import concourse.bass as bass
import concourse.tile as tile
from concourse import mybir
from concourse.bass_utils import run_bass_kernel_spmd
from concourse.vector_clock import ScopedClock

# ---------------------------------------------------------------- constants
N_NODES = 50000
IN_F = 256
HID = 64
HEADS = 4
OUT_F = 64
NEG_SLOPE = 0.2

NC_CORES = 8
P = 128
W_PER_CORE = 49
NWIN = NC_CORES * W_PER_CORE    # 392 windows globally
OWN = W_PER_CORE * P            # 6272 rows per core
PADN = NC_CORES * OWN           # 50176
F32 = mybir.dt.float32
BF = mybir.dt.bfloat16

EXEC_TIMES_NS = {}              # filled when GAT_PROFILE=1


# ------------------------------------------------------------- tile patches
def _patch_tile():
    """This container's walrus rejects instructions with >1 sem wait
    ("Too many sync wait commands").  After Tile lowering, move excess waits
    onto same-engine no-ops inserted before the offending instruction."""
    if getattr(_patch_tile, "done", False):
        return
    _patch_tile.done = True

    MAX_WAITS = 1

    def _split_all_waits(nc):
        for bb in nc.main_func.blocks:
            insts = bb.instructions
            i = 0
            while i < len(insts):
                inst = insts[i]
                si = getattr(inst, "sync_info", None)
                if si is None or len(si.on_wait) <= MAX_WAITS:
                    i += 1
                    continue
                waits = list(si.on_wait)
                si.on_wait[:] = waits[:MAX_WAITS]
                extra = waits[MAX_WAITS:]
                nops = []
                for j in range(0, len(extra), MAX_WAITS):
                    nop = mybir.InstNoOp(
                        name=f"I-waitsplit-{nc.next_id()}",
                        ins=[],
                        outs=[],
                        engine=inst.engine,
                    )
                    nop.sync_info = mybir.SyncInfo(
                        on_wait=extra[j : j + MAX_WAITS], on_update=[]
                    )
                    nc.register_instruction(nop, overwrite=True)
                    nops.append(nop)
                insts[i:i] = nops
                i += len(nops) + 1

    def _drain_and_barrier(self, tick_clock, wait_clock):
        drain_inst = self.nc.sync.drain()
        wait_clock.add_sem_waits(
            drain_inst.ins, ScopedClock({None: tick_clock.global_clock})
        )
        self.nc.all_engine_barrier()
        assert self.sems is not None
        popped = self.nc._tile_sem_poison_stack.pop()
        assert popped is self._sem_poison
        self.nc.clear_and_free_semaphores(list(self.sems.allocated().values()))
        self.nc.all_engine_barrier()
        _split_all_waits(self.nc)

    tile.TileContext._drain_and_barrier = _drain_and_barrier


def _install_ntff_hook():
    """Enable run_bass_kernel_spmd(trace=True) under axon: register the NTFF
    profile hook that the boot script skips when antenv.axon_hooks is absent."""
    if getattr(_install_ntff_hook, "done", False):
        return
    _install_ntff_hook.done = True
    try:
        mod = types.ModuleType("antenv.axon_hooks")
        _state = {}

        def set_axon_ntff_profile_hook(h):
            _state["h"] = h

        def get_axon_ntff_profile_hook():
            return _state.get("h")

        mod.set_axon_ntff_profile_hook = set_axon_ntff_profile_hook
        mod.get_axon_ntff_profile_hook = get_axon_ntff_profile_hook
        sys.modules["antenv.axon_hooks"] = mod
        import antenv

        antenv.axon_hooks = mod
        from trn_agent_boot.trn_boot import _ntff_profile_via_ctypes

        hook = _ntff_profile_via_ctypes("/opt/axon/libaxon_pjrt.so")
        if hook is not None:
            set_axon_ntff_profile_hook(hook)
    except Exception:
        pass


# ------------------------------------------------------------- kernel builders
def build_k1():
    """h|el|er table for this core's 6272 nodes: htab = xT_own^T @ W0T_ext.
    All bf16; PSUM f32 accumulate.  Output writes batched 7 windows/DMA."""
    nc = bass.Bass()
    DE = IN_F + 2 * HEADS                     # 264
    xT_own = nc.dram_tensor("xT_own", [IN_F, OWN], BF, kind="ExternalInput")
    w0te = nc.dram_tensor("w0te", [IN_F, DE], BF, kind="ExternalInput")
    htab = nc.dram_tensor("htab", [OWN, DE], BF, kind="ExternalOutput")

    WB = 7                                    # windows per batch (load + store)
    with tile.TileContext(nc) as tc:
        with (
            tc.tile_pool(name="const", bufs=1) as constp,
            tc.tile_pool(name="xin", bufs=3) as xinp,
            tc.tile_pool(name="out", bufs=3) as outp,
            tc.tile_pool(name="psum", bufs=3, space="PSUM") as psum,
        ):
            wt = constp.tile([P, 2, DE], BF)
            nc.sync.dma_start(wt[:, 0, :], w0te[0:P, :])
            nc.sync.dma_start(wt[:, 1, :], w0te[P : 2 * P, :])
            Q = WB * P
            for b in range(W_PER_CORE // WB):
                xt = xinp.tile([P, 2, Q], BF, tag="xt")
                for kk in range(2):
                    nc.sync.dma_start(
                        xt[:, kk, :],
                        xT_own[kk * P : (kk + 1) * P, b * Q : (b + 1) * Q],
                    )
                hsb = outp.tile([P, WB, DE], BF, tag="hsb")
                for i in range(WB):
                    pu = psum.tile([P, DE], F32, tag="pu")
                    for kk in range(2):
                        nc.tensor.matmul(
                            pu[:],
                            lhsT=xt[:, kk, i * P : (i + 1) * P],
                            rhs=wt[:, kk, :],
                            start=(kk == 0),
                            stop=(kk == 1),
                        )
                    nc.scalar.copy(hsb[:, i, :], pu[:])
                nc.scalar.dma_start(
                    htab[b * Q : (b + 1) * Q, :].rearrange("(i p) f -> p i f", p=P),
                    hsb[:],
                )
    return nc


def build_k2(C):
    """L0 edge phase + relu + L1 node matmul.

    Inputs (per core):
      h_edge [W, P, C*65*4] bf16  gathered h rows (src), (chunk, d, h)
                                  interleaved; d=64 row is ones (carries ee)
      meta   [W, P, C*9]    bf16  [0:C*8] el|er (chunk-outer, [c, 8]),
                                  [C*8:C*9] dstloc (chunk-minor, [c])
      iota   [P, 128*C]     bf16  iota[p, n*C+c] = n
      b0t    [P, 2]         f32   bias for transposed halves
      ident  [P, 128]       bf16
      w1te   [256, 66]      bf16  rows (d,h)-interleaved
    Output:
      g_out  [OWN, 66] bf16   g | el1 | er1 for this core's window slots
    """
    nc = bass.Bass()
    HF = HEADS * HID                           # 256
    G = OUT_F + 2                              # 66
    RW = HF + 4                                # 260 msg row count
    h_edge = nc.dram_tensor("h_edge", [W_PER_CORE, P, C * 65 * 4], BF, kind="ExternalInput")
    meta = nc.dram_tensor("meta", [W_PER_CORE, P, C * 9], BF, kind="ExternalInput")
    iota_t = nc.dram_tensor("iota", [P, 128 * C], BF, kind="ExternalInput")
    b0t = nc.dram_tensor("b0t", [P, 2], F32, kind="ExternalInput")
    ident_t = nc.dram_tensor("ident", [P, 128], BF, kind="ExternalInput")
    w1te = nc.dram_tensor("w1te", [HF, G], BF, kind="ExternalInput")
    g_out = nc.dram_tensor("g_out", [OWN, G], BF, kind="ExternalOutput")

    with tile.TileContext(nc) as tc:
        with (
            tc.tile_pool(name="const", bufs=1) as constp,
            tc.tile_pool(name="sbuf", bufs=5) as pool,
            tc.tile_pool(name="small", bufs=6) as spool,
            tc.tile_pool(name="psum", bufs=3, space="PSUM") as psum,
            tc.tile_pool(name="psumt", bufs=2, space="PSUM") as psumt,
            tc.tile_pool(name="psumg", bufs=2, space="PSUM") as psumg,
        ):
            b0_sb = constp.tile([P, 2], F32)
            nc.sync.dma_start(b0_sb[:], b0t[:])
            ident_sb = constp.tile([P, 128], BF)
            nc.sync.dma_start(ident_sb[:], ident_t[:])
            w1_sb = constp.tile([P, 2, G], BF)
            nc.sync.dma_start(w1_sb[:, 0, :], w1te[0:P, :])
            nc.sync.dma_start(w1_sb[:, 1, :], w1te[P : 2 * P, :])
            iota_sb = constp.tile([P, 128, C], BF)
            nc.sync.dma_start(iota_sb[:], iota_t[:].rearrange("p (n c) -> p n c", c=C))
            h1_all = constp.tile([P, W_PER_CORE * HF], BF)

            for w in range(W_PER_CORE):
                he = pool.tile([P, C, 65, 4], BF, tag="he")
                nc.sync.dma_start(
                    he[:], h_edge[w].rearrange("p (c j h) -> p c j h", j=65, h=4)
                )
                mt = spool.tile([P, C * 9], BF, tag="mt")
                nc.sync.dma_start(mt[:], meta[w])
                mA = mt[:, 0 : C * 8].rearrange("p (c k) -> p c k", k=8)
                mB = mt[:, C * 8 : C * 9]                     # [P, C] dstloc

                # one-hot S on DVE: S[p, n, c] = (dloc[p,c] == n)  (2x mode)
                S = pool.tile([P, 128, C], BF, tag="S")
                nc.vector.tensor_tensor(
                    out=S[:],
                    in0=mB[:, None, :].to_broadcast([P, 128, C]),
                    in1=iota_sb[:],
                    op=mybir.AluOpType.is_equal,
                )

                # e = el + er (DVE); leaky+exp on scalar engine
                e = spool.tile([P, C, 4], F32, tag="e")
                nc.vector.tensor_tensor(
                    out=e[:], in0=mA[:, :, 0:4], in1=mA[:, :, 4:8],
                    op=mybir.AluOpType.add,
                )
                nc.vector.scalar_tensor_tensor(
                    out=e[:], in0=e[:], scalar=NEG_SLOPE, in1=e[:],
                    op0=mybir.AluOpType.mult, op1=mybir.AluOpType.max,
                )
                ee = spool.tile([P, C, 4], BF, tag="ee")
                nc.scalar.activation(ee[:], e[:], mybir.ActivationFunctionType.Exp)

                # msg[p,c,j,h] = he * ee  (2x-mode DVE; j=64 row is ones -> ee)
                msg = pool.tile([P, C, 65, 4], BF, tag="msg")
                nc.vector.tensor_tensor(
                    out=msg[:],
                    in0=he[:],
                    in1=ee[:, :, None, :].to_broadcast([P, C, 65, 4]),
                    op=mybir.AluOpType.mult,
                )

                pu = psum.tile([P, RW], F32, tag="pu")
                for c in range(C):
                    nc.tensor.matmul(
                        pu[:], lhsT=S[:, :, c], rhs=msg[:, c],
                        start=(c == 0), stop=(c == C - 1),
                    )

                # normalize -> h1_all (bf16, pre-bias): per-head scalar copy
                # with per-partition scale rs[h].  No eps: every real dst has
                # deg >= 2 so s > 0; pad-slot rows (s=0 -> inf/NaN) are never
                # read downstream.
                rs = spool.tile([P, 4], F32, tag="rs")
                nc.vector.reciprocal(rs[:], pu[:, HF : HF + 4])
                h1w4 = h1_all[:, w * HF : (w + 1) * HF].rearrange(
                    "p (j h) -> p h j", h=4
                )
                pu4 = pu[:, 0:HF].rearrange("p (j h) -> p h j", h=4)
                for hd in range(HEADS):
                    nc.scalar.activation(
                        h1w4[:, hd, :], pu4[:, hd, :],
                        mybir.ActivationFunctionType.Copy,
                        scale=rs[:, hd : hd + 1],
                    )

                # L1 node matmul; bias + relu fold into the post-transpose
                # scalar copy (b0 is per-partition in transposed space)
                pg = psumg.tile([P, G], F32, tag="pg")
                for kk in range(2):
                    pt = psumt.tile([P, 128], BF, tag="pt")
                    nc.tensor.transpose(
                        out=pt[:],
                        in_=h1_all[:, w * HF + kk * P : w * HF + (kk + 1) * P],
                        identity=ident_sb[:],
                    )
                    h1t = spool.tile([P, 128], BF, tag="h1t")
                    nc.scalar.activation(
                        h1t[:], pt[:], mybir.ActivationFunctionType.Relu,
                        bias=b0_sb[:, kk : kk + 1],
                    )
                    nc.tensor.matmul(
                        pg[:], lhsT=h1t[:], rhs=w1_sb[:, kk, :],
                        start=(kk == 0), stop=(kk == 1),
                    )
                gsb = spool.tile([P, G], BF, tag="gsb")
                nc.scalar.copy(gsb[:], pg[:])
                nc.sync.dma_start(g_out[w * P : (w + 1) * P, :], gsb[:])
    return nc


def build_k3(d_list):
    """L1 edge phase, degree-sorted: one dst per partition, edges on free
    axis.  y = (sum_d ee1*g[src_d]) / (sum_d ee1) + b1.

    g_edge [P, TOT] bf16: per window w a [65, D_w] block at off_w:
      rows 0..63 = g feats of src, row 64 = el1[src] (pad slots = -300).
    er1t   [P, W] f32: er1 of this partition's dst, per window.
    """
    nc = bass.Bass()
    offs = np.concatenate([[0], np.cumsum([65 * d for d in d_list])])
    TOT = int(offs[-1])
    g_edge = nc.dram_tensor("g_edge", [P, TOT], BF, kind="ExternalInput")
    er1t = nc.dram_tensor("er1t", [P, W_PER_CORE], F32, kind="ExternalInput")
    b1r = nc.dram_tensor("b1r", [P, OUT_F], F32, kind="ExternalInput")
    y_out = nc.dram_tensor("y_out", [OWN, OUT_F], F32, kind="ExternalOutput")

    with tile.TileContext(nc) as tc:
        with (
            tc.tile_pool(name="const", bufs=1) as constp,
            tc.tile_pool(name="sbuf", bufs=6) as pool,
            tc.tile_pool(name="small", bufs=6) as spool,
        ):
            b1_sb = constp.tile([P, OUT_F], F32)
            nc.sync.dma_start(b1_sb[:], b1r[:])
            er_sb = constp.tile([P, W_PER_CORE], F32)
            nc.sync.dma_start(er_sb[:], er1t[:])

            GRP = 4
            w = 0
            while w < W_PER_CORE:
                ws = list(range(w, min(w + GRP, W_PER_CORE)))
                L = int(offs[ws[-1] + 1] - offs[w])
                ge = pool.tile([P, L], BF, tag="ge")
                nc.sync.dma_start(ge[:], g_edge[:, int(offs[w]) : int(offs[w]) + L])
                ysb = spool.tile([P, len(ws), OUT_F], F32, tag="ysb")
                for i, wi in enumerate(ws):
                    D = int(d_list[wi])
                    rel = int(offs[wi] - offs[w])
                    gew = ge[:, rel : rel + 65 * D].rearrange("p (j d) -> p j d", d=D)
                    # e2 = leaky(el1 + er1[dst]);  ee = exp(e2); s1 = sum_d ee
                    e2 = spool.tile([P, D], F32, tag=f"e2_{i}")
                    nc.vector.tensor_scalar_add(
                        e2[:], gew[:, 64, :], er_sb[:, wi : wi + 1]
                    )
                    nc.vector.scalar_tensor_tensor(
                        out=e2[:], in0=e2[:], scalar=NEG_SLOPE, in1=e2[:],
                        op0=mybir.AluOpType.mult, op1=mybir.AluOpType.max,
                    )
                    ee = spool.tile([P, D], BF, tag=f"ee_{i}")
                    s1 = spool.tile([P, 1], F32, tag=f"s1_{i}")
                    nc.scalar.activation(
                        ee[:], e2[:], mybir.ActivationFunctionType.Exp,
                        accum_out=s1[:],
                    )
                    msg = pool.tile([P, 64, D], BF, tag=f"msg_{i}")
                    nc.vector.tensor_tensor(
                        out=msg[:],
                        in0=gew[:, 0:64, :],
                        in1=ee[:, None, :].to_broadcast([P, 64, D]),
                        op=mybir.AluOpType.mult,
                    )
                    # pairwise fold (2x mode) then 1x reduce over the half
                    H1 = (D + 1) // 2
                    if D >= 6:
                        nc.vector.tensor_tensor(
                            out=msg[:, :, 0 : D - H1],
                            in0=msg[:, :, 0 : D - H1],
                            in1=msg[:, :, H1:D],
                            op=mybir.AluOpType.add,
                        )
                        red_in = msg[:, :, 0:H1]
                    else:
                        red_in = msg[:]
                    yagg = spool.tile([P, OUT_F], F32, tag=f"yagg_{i}")
                    nc.vector.tensor_reduce(
                        out=yagg[:], in_=red_in, axis=mybir.AxisListType.X,
                        op=mybir.AluOpType.add,
                    )
                    # s1 >= D*exp(-60) > 0 always (pad slots contribute), so
                    # the reciprocal needs no eps guard
                    rs = spool.tile([P, 1], F32, tag=f"rs_{i}")
                    nc.vector.reciprocal(rs[:], s1[:])
                    nc.vector.scalar_tensor_tensor(
                        out=ysb[:, i, :], in0=yagg[:], scalar=rs[:, 0:1],
                        in1=b1_sb[:],
                        op0=mybir.AluOpType.mult, op1=mybir.AluOpType.add,
                    )
                nc.scalar.dma_start(
                    y_out[w * P : (w + len(ws)) * P, :].rearrange(
                        "(i p) f -> p i f", p=P
                    ),
                    ysb[:],
                )
                w += GRP
    return nc


# ------------------------------------------------------------- host helpers
def _run(nc, in_maps, label):
    profile = os.environ.get("GAT_PROFILE", "0") == "1"
    res = run_bass_kernel_spmd(
        nc, in_maps, core_ids=list(range(NC_CORES)), trace=profile
    )
    if profile:
        EXEC_TIMES_NS[label] = res.exec_time_ns
    return res.results


def _balanced_windows(deg):
    """Snake-deal dsts (degree-desc) into NWIN windows of exactly 128.
    Returns wdst [NWIN, 128] node ids (-1 pad), win_of/loc_of [N]."""
    order = np.argsort(-deg, kind="stable")
    ids = np.full(NWIN * P, -1, dtype=np.int64)
    ids[: len(order)] = order
    rows = ids.reshape(P, NWIN)
    for r in range(1, P, 2):
        rows[r] = rows[r][::-1]
    wdst = rows.T.copy()                         # [NWIN, 128]

    # greedy swap pass: try to pull the max window load down to the next
    # chunk boundary by swapping single dsts with lighter windows
    def load_of(g):
        ids = wdst[g][wdst[g] >= 0]
        return deg[ids].sum() if len(ids) else 0

    loads = np.array([load_of(g) for g in range(NWIN)])
    target = ((loads.sum() + NWIN * P - 1) // (NWIN * P) + 0) * P
    target = max(target, int(np.ceil(loads.mean() / P)) * P)
    for _ in range(2000):
        a = int(np.argmax(loads))
        if loads[a] <= target:
            break
        b = int(np.argmin(loads))
        need = loads[a] - target              # reduce a by >= need
        da = deg[np.clip(wdst[a], 0, None)] * (wdst[a] >= 0)
        db = deg[np.clip(wdst[b], 0, None)] * (wdst[b] >= 0)
        diff = da[:, None] - db[None, :]      # swap gain for a
        ok = (diff >= need) & (loads[b] + diff <= target)
        if not ok.any():
            ok = (diff > 0) & (loads[b] + diff <= target)
            if not ok.any():
                break
        ia, ib = np.unravel_index(np.argmin(np.where(ok, diff, 1 << 30)), diff.shape)
        d = int(diff[ia, ib])
        wdst[a, ia], wdst[b, ib] = wdst[b, ib], wdst[a, ia]
        loads[a] -= d
        loads[b] += d

    win_of = np.empty(len(deg), dtype=np.int64)
    loc_of = np.empty(len(deg), dtype=np.int64)
    wi, li = np.nonzero(wdst >= 0)
    win_of[wdst[wi, li]] = wi
    loc_of[wdst[wi, li]] = li
    return wdst, win_of, loc_of


def kernel(x, src, dst, W0, al0, ar0, b0, W1, al1, ar1, b1):
    _patch_tile()
    _install_ntff_hook()

    import ml_dtypes

    BFH = ml_dtypes.bfloat16

    x = np.asarray(x, dtype=np.float32)
    src = np.asarray(src, dtype=np.int64)
    dst = np.asarray(dst, dtype=np.int64)
    W0 = np.asarray(W0, dtype=np.float32)
    al0 = np.asarray(al0, dtype=np.float32)
    ar0 = np.asarray(ar0, dtype=np.float32)
    b0 = np.asarray(b0, dtype=np.float32)
    W1 = np.asarray(W1, dtype=np.float32)
    al1 = np.asarray(al1, dtype=np.float32)
    ar1 = np.asarray(ar1, dtype=np.float32)
    b1 = np.asarray(b1, dtype=np.float32)

    DE = IN_F + 2 * HEADS
    HF = HEADS * HID
    G = OUT_F + 2
    E = len(src)

    # (d, h)-interleave permutation: new col j=(d*4+h) <- old col h*64+d
    perm = np.arange(HF).reshape(HEADS, HID).T.reshape(-1)

    # ---- weight prep (bf16)
    vl0 = np.einsum("hd,hdk->hk", al0, W0.reshape(HEADS, HID, IN_F))
    vr0 = np.einsum("hd,hdk->hk", ar0, W0.reshape(HEADS, HID, IN_F))
    w0te = np.concatenate([W0.T, vl0.T, vr0.T], axis=1).astype(BFH)    # [256, 264]
    vl1 = al1 @ W1
    vr1 = ar1 @ W1
    w1te = np.concatenate([W1.T, vl1.T, vr1.T], axis=1)[perm].astype(BFH)  # [256, 66]

    xT_pad = np.zeros((IN_F, PADN), dtype=BFH)
    xT_pad[:, :N_NODES] = x.T.astype(BFH)

    ident = np.eye(128, dtype=BFH)
    b0t = np.ascontiguousarray(b0[perm].reshape(2, P).T.astype(np.float32))
    b1r = np.tile(b1[None, :], (P, 1)).astype(np.float32)

    # ---- K1: node tables (node-id order)
    nc1 = build_k1()
    in1 = [
        {"xT_own": np.ascontiguousarray(xT_pad[:, k * OWN : (k + 1) * OWN]), "w0te": w0te}
        for k in range(NC_CORES)
    ]
    r1 = _run(nc1, in1, "k1")
    htab = np.concatenate([r1[k]["htab"] for k in range(NC_CORES)], axis=0)
    htab_x = np.concatenate([htab, np.zeros((1, DE), dtype=BFH)], axis=0)

    # ---- K2 window assignment (balanced)
    deg = np.bincount(dst, minlength=N_NODES)
    wdst, win_of, loc_of = _balanced_windows(deg)
    gwin = win_of[dst]                       # global window per edge
    dloc_e = loc_of[dst]

    order = np.argsort(gwin, kind="stable")
    s_src, s_gw, s_loc = src[order], gwin[order], dloc_e[order]
    cnt = np.bincount(s_gw, minlength=NWIN)
    C = int(np.ceil(cnt.max() / P))
    gstart = np.zeros(NWIN, dtype=np.int64)
    gstart[1:] = np.cumsum(cnt)[:-1]
    within = np.arange(E) - gstart[s_gw]

    # slot arrays [NWIN, C*P]
    sidx = np.full((NWIN, C * P), PADN, dtype=np.int64)
    ddst = np.full((NWIN, C * P), PADN, dtype=np.int64)
    dloc = np.full((NWIN, C * P), -1.0, dtype=np.float32)
    sidx[s_gw, within] = s_src
    ddst[s_gw, within] = dst[order]
    dloc[s_gw, within] = s_loc.astype(np.float32)
    # per-core views: core k owns globals k, k+8, ... (slot w = g // 8)
    core_of_g = np.arange(NWIN) % NC_CORES
    slot_of_g = np.arange(NWIN) // NC_CORES

    iota = np.broadcast_to(
        np.arange(128, dtype=np.float32)[:, None], (128, C)
    ).reshape(1, 128 * C)
    iota = np.ascontiguousarray(np.broadcast_to(iota, (P, 128 * C)).astype(BFH))

    # ---- K2 inputs
    nc2 = build_k2(C)
    ones_h = np.ones((W_PER_CORE, P, C, 1, HEADS), dtype=BFH)
    in2 = []
    for k in range(NC_CORES):
        gsel = np.nonzero(core_of_g == k)[0][np.argsort(slot_of_g[core_of_g == k])]
        sc = sidx[gsel]                              # [W, C*P]
        dc = ddst[gsel]
        hg = htab_x[sc, :HF]                         # [W, C*P, 256] bf16
        hg = hg.reshape(W_PER_CORE, C, P, HEADS, HID).transpose(0, 2, 1, 4, 3)
        h_edge = np.concatenate([hg, ones_h], axis=3).reshape(W_PER_CORE, P, -1)
        el = htab_x[sc, HF : HF + 4].reshape(W_PER_CORE, C, P, 4)
        er = htab_x[dc, HF + 4 : HF + 8].reshape(W_PER_CORE, C, P, 4)
        mA = np.concatenate([el, er], axis=3).transpose(0, 2, 1, 3)
        mB = dloc[gsel].reshape(W_PER_CORE, C, P).transpose(0, 2, 1)
        meta = np.concatenate(
            [mA.reshape(W_PER_CORE, P, C * 8), mB.astype(BFH)], axis=2
        )
        in2.append(
            {
                "h_edge": np.ascontiguousarray(h_edge),
                "meta": np.ascontiguousarray(meta.astype(BFH)),
                "iota": iota,
                "b0t": b0t,
                "ident": ident,
                "w1te": w1te,
            }
        )
    r2 = _run(nc2, in2, "k2")
    gtab = np.concatenate([r2[k]["g_out"] for k in range(NC_CORES)], axis=0)
    gtab_x = np.concatenate([gtab, np.zeros((1, G), dtype=BFH)], axis=0)
    # gtab row of node n: grow[n]
    grow = np.full(N_NODES + 1, PADN, dtype=np.int64)
    valid = wdst.reshape(-1) >= 0
    gflat = np.repeat(np.arange(NWIN), P)[valid]
    lflat = np.tile(np.arange(P), NWIN)[valid]
    grow[wdst.reshape(-1)[valid]] = (
        core_of_g[gflat] * OWN + slot_of_g[gflat] * P + lflat
    )

    # ---- K3: degree-sorted assignment
    order3 = np.argsort(-deg, kind="stable")         # node ids, deg desc
    ids3 = np.full(NWIN * P, -1, dtype=np.int64)
    ids3[:N_NODES] = order3
    wdst3 = ids3.reshape(NWIN, P)                    # window g3 -> 128 dsts
    Dg = np.maximum(deg[np.clip(wdst3[:, 0], 0, None)], 1)   # max deg per window
    Dg[wdst3[:, 0] < 0] = 1
    d_list = [int(Dg[w * NC_CORES]) for w in range(W_PER_CORE)]  # cross-core max

    # CSR of edges by dst
    eorder = np.argsort(dst, kind="stable")
    e_src_sorted = src[eorder]
    starts = np.zeros(N_NODES + 1, dtype=np.int64)
    starts[1:] = np.cumsum(deg)

    nc3 = build_k3(d_list)
    NEG = -300.0
    in3 = []
    for k in range(NC_CORES):
        blocks = []
        er1t = np.zeros((P, W_PER_CORE), dtype=np.float32)
        for w in range(W_PER_CORE):
            g3 = w * NC_CORES + k
            D = d_list[w]
            dsts = wdst3[g3]                          # [128] node ids (-1 pad)
            dcl = np.clip(dsts, 0, None)
            dgs = np.where(dsts >= 0, deg[dcl], 0)
            mask = np.arange(D)[None, :] < dgs[:, None]            # [128, D]
            eidx = starts[dcl][:, None] + np.arange(D)[None, :]
            srcs = np.where(mask, e_src_sorted[np.clip(eidx, 0, E - 1)], -1)
            rows = grow[np.where(srcs >= 0, srcs, N_NODES)]        # gtab rows
            blk = np.empty((P, 65, D), dtype=BFH)
            blk[:, 0:64, :] = gtab_x[rows, :OUT_F].transpose(0, 2, 1)
            el1 = gtab_x[rows, OUT_F].astype(np.float32)
            el1[~mask] = NEG
            blk[:, 64, :] = el1.astype(BFH)
            blocks.append(blk.reshape(P, 65 * D))
            er1t[:, w] = np.where(
                dsts >= 0, gtab_x[grow[dcl], OUT_F + 1].astype(np.float32), 0.0
            )
        in3.append(
            {
                "g_edge": np.ascontiguousarray(np.concatenate(blocks, axis=1)),
                "er1t": er1t,
                "b1r": b1r,
            }
        )
    r3 = _run(nc3, in3, "k3")
    yout = np.concatenate([r3[k]["y_out"] for k in range(NC_CORES)], axis=0)
    # unpermute: row (k, w*128+p) = node wdst3[w*8+k, p]
    y = np.zeros((N_NODES, OUT_F), dtype=np.float32)
    for k in range(NC_CORES):
        g3s = np.arange(W_PER_CORE) * NC_CORES + k
        nodes = wdst3[g3s].reshape(-1)                # [OWN]
        rows = yout[k * OWN : (k + 1) * OWN]
        ok = nodes >= 0
        y[nodes[ok]] = rows[ok]
    return np.ascontiguousarray(y)
